# revision 39
# baseline (speedup 1.0000x reference)
"""Causal self-attention (B=2, T=2048, C=2048, H=16, D=128) on 8 TRN2 NeuronCores.

Sharding: 8 cores = 2 batches x 4 head-groups (4 heads each).
Core m: batch b = m // 4, heads [4g, 4g+4) with g = m % 4.
  - c_attn columns split by head (tensor parallel), c_proj rows split by head.
  - Each core returns a partial projection output (bf16); host sums the 4
    partials per batch in f32 and adds b_proj.

All matmul operands are bf16 (full PE speed AND fast weight load, so
LDWEIGHTS hides under the matmul stream). PSUM accumulation stays f32.
x is pre-transposed on the host, so xT tiles load as plain contiguous DMAs,
batched into a few large transfers (per-DMA dispatch on the sync queue costs
~0.6us, so many small DMAs throttle the front of phase A).

Per-core pipeline:
  A:  QT/KT = (x @ W)^T accumulated in PSUM (moving = xT chunks),
      evacuated bf16 into SBUF-resident per-head tiles.
      V computed directly in [token, dim] orientation (stationary = xT
      slices, moving = all-heads wv chunk) -> 16 resident [128 t, 512 d]
      tiles; bias added during the DVE evacuation.
  B:  per head: ST = K Q^T chunk pairs -> exp (bf16) -> (mask-mul on
      diagonal pairs) -> yT += V_chunk^T @ PT, sums += ones^T @ PT;
      1/sum on the DVE (reciprocal_approx_fast) so the ACT engine only
      ever runs Exp (an Exp<->Ln switch costs a 1.3us table reload that
      serializes the softmax chain); emission software-pipelined
      (chunk-pair lag) so the PE never queues behind ACT.
  C:  out = concat_heads(y) @ Wp_rows (partial, bf16) -> DRAM output
"""
import sys

sys.path.insert(0, "/opt/trn_rl_repo")
sys.path.insert(0, "/root/.axon_site")

import numpy as np

N_EMBD = 2048
N_HEAD = 16
HEAD_DIM = 128
B, T = 2, 2048
N_CORES = 8
H_PER_CORE = 4          # heads per core
HD = H_PER_CORE * HEAD_DIM  # 512: per-core q/k/v width
NC_C = N_EMBD // 128    # 16 contraction chunks
NT = T // 128           # 16 token 128-blocks
NQB = T // 512          # 4 q blocks of 512
SCALE = 1.0 / np.sqrt(HEAD_DIM)

# xT chunk groups per DMA: leading chunks fine-grained so the first
# matmuls start early, trailing chunks batched for cheap dispatch
XT_GROUPS = [(0, 1), (1, 2), (3, 3), (6, 4), (10, 6)]
WQ_GROUPS = [(0, 2), (2, 6), (8, 8)]

_CACHE = {}


def _build():
    import concourse.bacc as bacc
    import concourse.mybir as mybir
    import concourse.tile as tile

    f32 = mybir.dt.float32
    bf16 = mybir.dt.bfloat16
    Exp = mybir.ActivationFunctionType.Exp
    Ident = mybir.ActivationFunctionType.Identity

    nc = bacc.Bacc("TRN2", target_bir_lowering=False, debug=False, num_devices=N_CORES)

    xt_dram = nc.dram_tensor("xt", [N_EMBD, T], bf16, kind="ExternalInput").ap()
    wq_dram = nc.dram_tensor("wq", [N_EMBD, HD], bf16, kind="ExternalInput").ap()
    wk_dram = nc.dram_tensor("wk", [N_EMBD, HD], bf16, kind="ExternalInput").ap()
    wv_dram = nc.dram_tensor("wv", [N_EMBD, HD], bf16, kind="ExternalInput").ap()
    bq_dram = nc.dram_tensor("bq", [HD, 1], f32, kind="ExternalInput").ap()
    bk_dram = nc.dram_tensor("bk", [HD, 1], f32, kind="ExternalInput").ap()
    bvb_dram = nc.dram_tensor("bvb", [128, HD], bf16, kind="ExternalInput").ap()
    wp_dram = nc.dram_tensor("wp", [HD, N_EMBD], bf16, kind="ExternalInput").ap()
    ones_dram = nc.dram_tensor("ones", [128, 128], bf16, kind="ExternalInput").ap()
    mmask_dram = nc.dram_tensor("mmask", [128, 2, 1024], bf16, kind="ExternalInput").ap()
    out_dram = nc.dram_tensor("out", [T, N_EMBD], bf16, kind="ExternalOutput").ap()

    with tile.TileContext(nc) as tc:
        with tc.tile_pool(name="singles", bufs=1) as singles, \
             tc.tile_pool(name="qk", bufs=1) as qk_pool, \
             tc.tile_pool(name="vres", bufs=1) as v_pool:

            # per-head SBUF-resident Q^T / K^T tiles [128 d, T]
            qt_t = [qk_pool.tile([128, T], bf16, tag=f"qt{h}", name=f"qt{h}")
                    for h in range(H_PER_CORE)]
            kt_t = [qk_pool.tile([128, T], bf16, tag=f"kt{h}", name=f"kt{h}")
                    for h in range(H_PER_CORE)]
            # V resident tiles: per token-block [128 t, 512 d(all heads)]
            v_t = [v_pool.tile([128, HD], bf16, tag=f"v{tb}", name=f"v{tb}")
                   for tb in range(NT)]

            bias_t = singles.tile([128, 2 * H_PER_CORE], f32)
            # full [128,128] ones stationary: a [1,512] sum output uses a
            # single PE column group and its drain adds ~93ns to the next
            # matmul; a [128,512] output drains normally and doubles as the
            # broadcast of the softmax denominator
            ones_sq = singles.tile([128, 128], bf16)
            bv_bc = singles.tile([128, HD], bf16)

            # ---------------- Phase A ----------------
            with tc.tile_pool(name="xt", bufs=1) as xt_pool, \
                 tc.tile_pool(name="wqkv", bufs=1) as wqkv_pool:
                # xT in grouped tiles; chunk c -> (tile, local index)
                xt_tiles = {}
                xt = []
                wq_g = []
                for gi, (c0, ng) in enumerate(XT_GROUPS):
                    gt = xt_pool.tile([128, ng, T], bf16, tag=f"xtg{gi}", name=f"xtg{gi}")
                    src = xt_dram.rearrange("(c p) t -> p c t", p=128)[:, c0:c0 + ng, :]
                    if gi == 0:
                        # 4 sub-transfers: the very first matmul only needs the
                        # first 512 tokens of chunk 0
                        for tq in range(4):
                            nc.sync.dma_start(gt[:, :, tq * 512:(tq + 1) * 512],
                                              src[:, :, tq * 512:(tq + 1) * 512])
                    else:
                        nc.sync.dma_start(gt[:], src)
                    for j in range(ng):
                        xt.append(gt[:, j, :])
                    # interleave wq pieces early: a tiny first piece so the
                    # very first matmul isn't gated on a 1MB transfer, and
                    # the rest before the od pair reaches those chunks
                    if gi < len(WQ_GROUPS):
                        wc0, wng = WQ_GROUPS[gi]
                        w = wqkv_pool.tile([128, wng, HD], bf16, tag=f"wq{gi}",
                                           name=f"wq{gi}")
                        nc.sync.dma_start(
                            w[:], wq_dram.rearrange("(c p) d -> p c d", p=128)[
                                :, wc0:wc0 + wng, :])
                        wq_g.append((wc0, wng, w))

                w_groups = {0: wq_g}
                for kind, src_dram in ((1, wk_dram), (2, wv_dram)):
                    gs = []
                    for half in range(2):
                        w = wqkv_pool.tile([128, 8, HD], bf16, tag=f"w{kind}_{half}",
                                           name=f"w{kind}_{half}")
                        nc.sync.dma_start(
                            w[:], src_dram.rearrange("(c p) d -> p c d", p=128)[
                                :, half * 8:(half + 1) * 8, :])
                        gs.append((half * 8, 8, w))
                    w_groups[kind] = gs

                def w_chunk(kind, c):
                    for c0_, ng_, t in w_groups[kind]:
                        if c0_ <= c < c0_ + ng_:
                            return t[:, c - c0_, :]
                    raise KeyError((kind, c))

                nc.sync.dma_start(bias_t[:, 0:4], bq_dram.rearrange("(a p) o -> p (a o)", p=128))
                nc.sync.dma_start(bias_t[:, 4:8], bk_dram.rearrange("(a p) o -> p (a o)", p=128))
                nc.sync.dma_start(ones_sq[:], ones_dram[:])
                nc.sync.dma_start(bv_bc[:], bvb_dram[:])

                # A-qk: QT/KT (transposed orientation) -> resident SBUF tiles.
                # od groups run in PAIRS (8 psum banks = 2 od x 4 tqb): doubles
                # the PE work per arriving xT byte, so the first pass keeps
                # pace with the DMA fill instead of stalling chunk by chunk.
                with tc.tile_pool(name="psA2", bufs=1, space="PSUM") as psA2:
                    for kind in range(2):
                        for pair in range(H_PER_CORE // 2):
                            ods = (2 * pair, 2 * pair + 1)
                            psums = {}
                            for oi, od_l in enumerate(ods):
                                for tqb in range(NQB):
                                    psums[(od_l, tqb)] = psA2.tile(
                                        [128, 512], f32, tag=f"qk{oi * NQB + tqb}",
                                        name=f"qk{kind}_{od_l}_{tqb}")
                            for c in range(NC_C):
                                wc = w_chunk(kind, c)
                                for od_l in ods:
                                    for tqb in range(NQB):
                                        nc.tensor.matmul(
                                            psums[(od_l, tqb)][:],
                                            wc[:, od_l * 128:(od_l + 1) * 128],
                                            xt[c][:, tqb * 512:(tqb + 1) * 512],
                                            start=(c == 0), stop=(c == NC_C - 1),
                                        )
                            for od_l in ods:
                                od = kind * H_PER_CORE + od_l  # bias column index
                                dst = (qt_t, kt_t)[kind][od_l]
                                for tqb in range(NQB):
                                    nc.scalar.activation(
                                        dst[:, tqb * 512:(tqb + 1) * 512],
                                        psums[(od_l, tqb)][:], bias=bias_t[:, od:od + 1],
                                        func=Ident, scale=1.0)

                    # A-v: V in [token, dim] orientation -> resident tiles;
                    # bias added during the DVE evacuation (it varies along
                    # the free dim here, which the ACT bias port can't do)
                    for tb in range(NT):
                        pv = psA2.tile([128, HD], f32, tag=f"qk{tb % 8}", name=f"pv{tb}")
                        for c in range(NC_C):
                            nc.tensor.matmul(
                                pv[:], xt[c][:, tb * 128:(tb + 1) * 128],
                                w_chunk(2, c),
                                start=(c == 0), stop=(c == NC_C - 1),
                            )
                        with nc.allow_low_precision(reason="v evac + bias bf16"):
                            nc.vector.tensor_add(v_t[tb][:], pv[:], bv_bc[:])

            # ---------------- Phases B & C ----------------
            with tc.tile_pool(name="ytc", bufs=1) as ytc_pool, \
                 tc.tile_pool(name="wp", bufs=1) as wp_pool, \
                 tc.tile_pool(name="bconst", bufs=1) as bconst:
                ytc = []  # resident normalized y^T tiles [128 d, 512 q] per (h, qb)
                for i in range(H_PER_CORE * NQB):
                    t = ytc_pool.tile([128, 512], bf16, tag=f"ytc{i}", name=f"ytc{i}")
                    ytc.append(t)
                wp_t = []
                mmask = bconst.tile([128, 2, 1024], bf16)

                with tc.tile_pool(name="pt", bufs=8) as pt_pool, \
                     tc.tile_pool(name="ptm", bufs=4) as ptm_pool, \
                     tc.tile_pool(name="small", bufs=2) as small_pool, \
                     tc.tile_pool(name="psB", bufs=2, space="PSUM") as psB, \
                     tc.tile_pool(name="psB1", bufs=1, space="PSUM") as psB1:
                    deferred = []  # emission closures, flushed with a lag
                    rinv_box = {}

                    def flush(keep):
                        while len(deferred) > keep:
                            deferred.pop(0)()

                    nc.sync.dma_start(mmask[:], mmask_dram[:])
                    for h in range(H_PER_CORE):
                        w = wp_pool.tile([128, N_EMBD], bf16, tag=f"wp{h}", name=f"wp{h}")
                        nc.sync.dma_start(w[:], wp_dram[h * 128:(h + 1) * 128, :])
                        wp_t.append(w)

                    for h in range(H_PER_CORE):
                        kt_h, qt_h = kt_t[h], qt_t[h]
                        hs = h * 128

                        for qb in reversed(range(NQB)):
                            i = h * NQB + qb
                            nkc = 4 * (qb + 1)
                            flush(keep=2)
                            yt_ps = psB.tile([128, 512], f32, tag="yt", name=f"yt{h}_{qb}", bufs=2)
                            sum_ps = psB1.tile([128, 512], f32, tag="sum", name=f"sum{h}_{qb}",
                                               bufs=2)
                            qs = qb * 512

                            # segments: below-diagonal kc pairs at full width,
                            # then the 4 diagonal kc at trimmed q-ranges
                            # (kc 4qb+j only reaches q-offset >= 128j):
                            # widths 512/384/256/128 - saves ~30% of B columns
                            segs = [([(2 * kp, 0, 512), (2 * kp + 1, 0, 512)], 1024, None)
                                    for kp in range(2 * qb)]
                            segs.append(([(4 * qb, 0, 512), (4 * qb + 1, 128, 384)],
                                         896, mmask[:, 0, 0:896]))
                            segs.append(([(4 * qb + 2, 256, 256), (4 * qb + 3, 384, 128)],
                                         384, mmask[:, 1, 0:384]))

                            for si, (cols, w, mask) in enumerate(segs):
                                st_f = psB.tile([128, 1024], f32, tag="st", name=f"st{h}_{qb}_{si}")
                                st = st_f[:, 0:w]
                                off = 0
                                lay = []  # (kc, q-offset, width, st-offset)
                                for kc, qo, kw in cols:
                                    nc.tensor.matmul(
                                        st[:, off:off + kw],
                                        kt_h[:, kc * 128:(kc + 1) * 128],
                                        qt_h[:, qs + qo:qs + qo + kw],
                                        start=True, stop=True,
                                    )
                                    lay.append((kc, qo, kw, off))
                                    off += kw
                                pt_f = pt_pool.tile([128, 1024], bf16, tag="pt",
                                                    name=f"pt{h}_{qb}_{si}")
                                pt = pt_f[:, 0:w]
                                nc.scalar.activation(pt[:], st[:], Exp, scale=SCALE)
                                if mask is not None:
                                    ptm_f = ptm_pool.tile([128, 1024], bf16, tag="ptm",
                                                          name=f"ptm{h}_{qb}_{si}")
                                    ptm = ptm_f[:, 0:w]
                                    with nc.allow_low_precision(reason="causal mask mul bf16"):
                                        nc.vector.tensor_mul(ptm[:], pt[:], mask)
                                    src = ptm
                                else:
                                    src = pt

                                def consume(src=src, yt_ps=yt_ps, sum_ps=sum_ps, lay=lay,
                                            nkc=nkc, hs=hs, h_=h, qb_=qb,
                                            last=(si == len(segs) - 1)):
                                    for kc2, qo, kw, off in lay:
                                        nc.tensor.matmul(
                                            yt_ps[:, qo:qo + kw], v_t[kc2][:, hs:hs + 128],
                                            src[:, off:off + kw],
                                            start=(kc2 == 0), stop=(kc2 == nkc - 1),
                                            skip_group_check=True,
                                        )
                                        nc.tensor.matmul(
                                            sum_ps[:, qo:qo + kw], ones_sq[:],
                                            src[:, off:off + kw],
                                            start=(kc2 == 0), stop=(kc2 == nkc - 1),
                                            skip_group_check=True,
                                        )
                                    if last:
                                        # 1/sum on the DVE, on the already
                                        # partition-broadcast [128,512] sums:
                                        # keeps Ln off the ACT engine (table
                                        # reloads serialize it) and replaces
                                        # the ones-row broadcast matmul
                                        ri32 = small_pool.tile([128, 512], f32, tag="ri32",
                                                               name=f"r32{h_}_{qb_}")
                                        nc.vector.reciprocal_approx_fast(ri32[:], sum_ps[:])
                                        rinv_box[(h_, qb_)] = ri32

                                deferred.append(consume)
                                flush(keep=4)

                            def norm(i=i, yt_ps=yt_ps, h_=h, qb_=qb):
                                rinv = rinv_box.pop((h_, qb_))
                                with nc.allow_low_precision(reason="softmax normalize bf16"):
                                    nc.vector.tensor_mul(ytc[i][:], yt_ps[:], rinv[:])

                            deferred.append(norm)
                            flush(keep=4)
                    flush(keep=0)

                # ---------------- Phase C ----------------
                with tc.tile_pool(name="oev", bufs=2) as oev_pool, \
                     tc.tile_pool(name="psC", bufs=2, space="PSUM") as psC:
                    for tb in reversed(range(NT)):
                        qb, ts = tb // 4, (tb % 4) * 128
                        oev = oev_pool.tile([128, N_EMBD], bf16, tag="oev", name=f"oev{tb}")
                        for ob in range(4):
                            po = psC.tile([128, 512], f32, tag=f"po{ob % 2}", name=f"po{tb}_{ob}")
                            for h in range(H_PER_CORE):
                                nc.tensor.matmul(
                                    po[:], ytc[h * NQB + qb][:, ts:ts + 128],
                                    wp_t[h][:, ob * 512:(ob + 1) * 512],
                                    start=(h == 0), stop=(h == H_PER_CORE - 1),
                                )
                            if ob % 2 == 0:
                                nc.scalar.copy(oev[:, ob * 512:(ob + 1) * 512], po[:])
                            else:
                                with nc.allow_low_precision(reason="out evac bf16"):
                                    nc.vector.tensor_copy(oev[:, ob * 512:(ob + 1) * 512], po[:])
                        if tb < 2:
                            # last iterations: split the store so the tail
                            # transfer overlaps the remaining evacuations
                            for hh_ in range(2):
                                nc.sync.dma_start(
                                    out_dram[tb * 128:(tb + 1) * 128,
                                             hh_ * 1024:(hh_ + 1) * 1024],
                                    oev[:, hh_ * 1024:(hh_ + 1) * 1024])
                        else:
                            nc.sync.dma_start(out_dram[tb * 128:(tb + 1) * 128, :], oev[:])

    nc.compile()
    return nc


def _consts():
    import ml_dtypes
    bf = ml_dtypes.bfloat16
    # trimmed-diagonal masks: each diagonal kc j covers q-offsets [128j, 512)
    # of its q-block; only the leading 128 columns of each kc's range are a
    # triangle, the rest pass through.
    tri = np.where(np.arange(128)[None, :] >= np.arange(128)[:, None], 1.0, 0.0)
    on = np.ones((128, 128), dtype=np.float64)
    mmask = np.zeros((128, 2, 1024), dtype=np.float32)
    # segment d0: kc 4qb (512 wide: tri + 3x ones) | kc 4qb+1 (384: tri + 2x ones)
    mmask[:, 0, 0:896] = np.concatenate([tri, on, on, on, tri, on, on], axis=1)
    # segment d1: kc 4qb+2 (256: tri + ones) | kc 4qb+3 (128: tri)
    mmask[:, 1, 0:384] = np.concatenate([tri, on, tri], axis=1)
    return {
        "ones": np.ones((128, 128), bf),
        "mmask": mmask.astype(bf),
    }


def _run(inputs, trace=False):
    import ml_dtypes
    from concourse.bass_utils import run_bass_kernel_spmd

    bf = ml_dtypes.bfloat16
    if "nc" not in _CACHE:
        _CACHE["nc"] = _build()
    nc = _CACHE["nc"]

    x = np.asarray(inputs["x"], dtype=np.float32)
    W_attn = np.asarray(inputs["W_attn"], dtype=np.float32)
    b_attn = np.asarray(inputs["b_attn"], dtype=np.float32)
    W_proj = np.asarray(inputs["W_proj"], dtype=np.float32)
    b_proj = np.asarray(inputs["b_proj"], dtype=np.float32)

    xtb = [np.ascontiguousarray(x[b].T.astype(bf)) for b in range(B)]
    consts = _consts()
    in_maps = []
    for m in range(N_CORES):
        b, g = m // 4, m % 4
        cs = g * HD
        im = {
            "xt": xtb[b],
            "wq": np.ascontiguousarray(W_attn[:, cs:cs + HD].astype(bf)),
            "wk": np.ascontiguousarray(W_attn[:, N_EMBD + cs:N_EMBD + cs + HD].astype(bf)),
            "wv": np.ascontiguousarray(W_attn[:, 2 * N_EMBD + cs:2 * N_EMBD + cs + HD].astype(bf)),
            "bq": np.ascontiguousarray(b_attn[cs:cs + HD].reshape(HD, 1)),
            "bk": np.ascontiguousarray(b_attn[N_EMBD + cs:N_EMBD + cs + HD].reshape(HD, 1)),
            "bvb": np.ascontiguousarray(np.broadcast_to(
                b_attn[2 * N_EMBD + cs:2 * N_EMBD + cs + HD].reshape(1, HD),
                (128, HD)).astype(bf)),
            "wp": np.ascontiguousarray(W_proj[cs:cs + HD, :].astype(bf)),
        }
        im.update(consts)
        in_maps.append(im)

    res = run_bass_kernel_spmd(nc, in_maps, list(range(N_CORES)), trace=trace)
    out = np.zeros((B, T, N_EMBD), dtype=np.float32)
    for m in range(N_CORES):
        out[m // 4] += res.results[m]["out"].astype(np.float32)
    out += b_proj
    return out, res


def kernel(**inputs) -> np.ndarray:
    out, _ = _run(inputs, trace=False)
    return out
